# revision 24
# baseline (speedup 1.0000x reference)
"""Trainium2 Bass kernel for the non-local (self-attention over spatial
positions) block.

Per batch b (8 batches -> one per NeuronCore):
    xf    = x[b]                       [C=128, N=4096]
    theta = w_theta @ xf               [64, N]
    phi   = w_phi   @ xf               [64, N]
    g     = w_g     @ xf               [64, N]
    attn  = softmax(theta^T phi)       [N, N]   (softmax over keys m)
    y     = g @ attn^T                 [64, N]
    out   = w_last @ y + xf            [128, N]

Design (per core):
 - scoresT orientation: scoresT[m, q] = sum_k phi[k,m] theta[k,q] with phi
   m-tiles stationary; exp(scoresT) feeds the y matmul directly as the
   moving operand (no transposes).
 - exp is the single-engine bottleneck (N*N = 16.7M elems/core) so it is
   SPLIT across two engines: 17/32 m-tiles per chunk on ACT (table exp,
   bf16 out) and 15/32 on DVE via a Schraudolph bit-trick: the bf16
   bits of ~exp(x) are round(128*log2e*x + B16), computed by one
   tensor_scalar (f32 PSUM -> int16 SBUF) and bitcast to bf16. The
   approximation's constant scale factor cancels in softmax.
 - q is processed in 4 big-chunks of 1024 (as two 512 halves qA/qB so
   each score matmul's PSUM output stays within one bank): one phi
   stationary serves two 512 streams, exp runs one 1024-wide call per
   m-tile, and y matmuls consume 512 halves. This keeps PE busy-fraction
   high enough for the HAM clock gate to hold 2.4 GHz.
 - theta/phi/x-projection operands are fp16 (10-bit mantissa, noise
   comparable to f32r) so all LDWEIGHTS get fast-weight-load.
 - One FLAT software pipeline across big-chunks: the PE queue is strict
   FIFO, so y matmuls are issued YLAG steps behind their score matmuls
   (covering the exp latency) and epilogues are split in two stages.
 - No max-subtraction: logits are within +-75; exp fits f32/bf16 range
   and the bit-trick constants are valid to |x|<88.
 - Row sums via a ones column appended to gT; reciprocal via the fast
   custom-DVE approx; partition-broadcast and residual-add on GPSIMD.
"""

import sys

import numpy as np

for _p in ("/opt/trn_rl_repo",):
    if _p not in sys.path:
        sys.path.insert(0, _p)

import concourse.bass as bass
from concourse import bacc
import concourse.mybir as mybir
import concourse.tile as tile
from concourse.bass_utils import run_bass_kernel_spmd

F32 = mybir.dt.float32
F32R = mybir.dt.float32r
BF16 = mybir.dt.bfloat16
FP16 = mybir.dt.float16
I16 = mybir.dt.int16

P = 128     # channels C / partition dim
CB = 64     # bottleneck channels
NQ = 4096   # spatial positions (64*64)
MT = 32     # m (key) tiles of 128
YLAG = 3    # y matmuls trail the score matmuls by this many big-steps
OLAG = 3    # out-projection trails the chunk's last y matmul

LOG2E = 1.4426950408889634
S16 = 128.0 * LOG2E
B16 = 127.0 * 128.0 - 5.60   # Schraudolph bias tuned for min max-rel-err

_NC_CACHE = {}


def _dve_mtile(mi):
    """Which m-tiles run their exp on the DVE (15 of 32; ACT gets 17
    plus the per-chunk epilogue copies)."""
    return mi % 2 == 1 and mi != 31


def _build():
    nc = bacc.Bacc()
    x_in = nc.declare_dram_parameter("xb", [P, NQ], F32, isOutput=False)
    xh_in = nc.declare_dram_parameter("xh", [P, NQ], FP16, isOutput=False)
    wqa_in = nc.declare_dram_parameter("wqa", [P, P], FP16, isOutput=False)
    wqb_in = nc.declare_dram_parameter("wqb", [P, P], FP16, isOutput=False)
    wg_in = nc.declare_dram_parameter("wgT", [P, CB], FP16, isOutput=False)
    wl_in = nc.declare_dram_parameter("wl", [CB, P], F32, isOutput=False)
    out_d = nc.declare_dram_parameter("out", [P, NQ], F32, isOutput=True)

    with tile.TileContext(nc) as tc:
        with (
            tc.tile_pool(name="const", bufs=1) as const,
            tc.tile_pool(name="big", bufs=1) as big,
            tc.tile_pool(name="work", bufs=2) as work,
            tc.tile_pool(name="probs", bufs=8) as probs,
            tc.tile_pool(name="spool", bufs=3, space="PSUM") as spool,
            tc.tile_pool(name="ypool", bufs=1, space="PSUM") as ypool,
        ):
            # ---- loads: fp16 x first (feeds all projections), f32 x
            # later (residual only); spread across two DMA queues ----
            xh = big.tile([P, NQ], FP16)
            for j in range(4):
                js = slice(j * 1024, (j + 1) * 1024)
                eng = nc.sync if j % 2 == 0 else nc.gpsimd
                eng.dma_start(out=xh[:, js], in_=xh_in[:, js])
            wqa = const.tile([P, P], FP16)
            wqb = const.tile([P, P], FP16)
            wg = const.tile([P, CB], FP16)
            wl = const.tile([CB, P], F32)
            nc.sync.dma_start(out=wqa, in_=wqa_in[:, :])
            nc.gpsimd.dma_start(out=wqb, in_=wqb_in[:, :])
            nc.sync.dma_start(out=wg, in_=wg_in[:, :])
            nc.gpsimd.dma_start(out=wl, in_=wl_in[:, :])
            wlr = const.tile([CB, P], F32R)
            nc.vector.tensor_copy(wlr, wl)
            xb = big.tile([P, NQ], F32)
            for j in range(8):
                js = slice(j * 512, (j + 1) * 512)
                eng = nc.sync if j % 2 == 0 else nc.gpsimd
                eng.dma_start(out=xb[:, js], in_=x_in[:, js])

            # gT in 65-col slots (col 64 = ones for the row-sum trick);
            # 8 m-tiles batched per PSUM slot; these small matmuls also
            # warm the PE's HAM clock gate before the projections
            gt = big.tile([P, MT * (CB + 1)], BF16)
            nc.vector.memset(gt, 1.0)
            gt3 = gt.rearrange("p (m c) -> p m c", c=CB + 1)
            for b2 in range(2):
                gp = spool.tile([P, 1024], F32, tag="s")
                gp3 = gp.rearrange("p (m c) -> p m c", c=CB)
                for k in range(16):
                    mi = b2 * 16 + k
                    nc.tensor.matmul(
                        gp3[:, k, :], xh[:, mi * 128:(mi + 1) * 128], wg,
                        start=True, stop=True,
                    )
                nc.scalar.copy(
                    gt3[:, b2 * 16:(b2 + 1) * 16, 0:CB], gp3[:, :, :]
                )

            # ---- projections: wqa/wqb = [w^T | w^T] duplicate theta/phi
            # into both row halves so score matmuls for two m-tiles run
            # concurrently in disjoint PE row groups. phi copies on ACT,
            # theta copies on DVE. ----
            theta = big.tile([P, NQ], FP16)
            phi = big.tile([P, NQ], FP16)
            for j in range(4):
                js = slice(j * 1024, (j + 1) * 1024)
                pp = spool.tile([P, 1024], F32, tag="s")
                for v in range(2):
                    vs = slice(j * 1024 + v * 512, j * 1024 + (v + 1) * 512)
                    nc.tensor.matmul(pp[:, v * 512:(v + 1) * 512],
                                     wqb, xh[:, vs], start=True, stop=True)
                nc.scalar.copy(phi[:, js], pp)
                pt = spool.tile([P, 1024], F32, tag="s")
                for v in range(2):
                    vs = slice(j * 1024 + v * 512, j * 1024 + (v + 1) * 512)
                    nc.tensor.matmul(pt[:, v * 512:(v + 1) * 512],
                                     wqa, xh[:, vs], start=True, stop=True)
                nc.vector.tensor_copy(theta[:, js], pt)

            # ---- flat main pipeline: 4 big-chunks x 16 m-pair steps ----
            yps_t = {}      # Qc -> y accumulator tile [65, 1024]
            pend_y = []     # (step, Qc, i, [pb0, pb1])
            pend_ep = []    # (due_step, args)

            def y_mm(Qc, i, pbh):
                for h in range(2):
                    mi = 2 * i + h
                    for v in range(2):
                        nc.tensor.matmul(
                            yps_t[Qc][:, v * 512:(v + 1) * 512],
                            gt[:, mi * (CB + 1):(mi + 1) * (CB + 1)],
                            pbh[h][:, v * 512:(v + 1) * 512],
                            start=(i == 0 and h == 0),
                            stop=(i == 15 and h == 1),
                        )

            def epilogue_a(Qc):
                # right after the chunk's last y matmul: drain PSUM
                yu = work.tile([CB + 1, 1024], F32R, tag="yu")
                nc.scalar.copy(yu, yps_t[Qc])
                ys = work.tile([1, 1024], F32, tag="ys")
                nc.scalar.copy(ys, yps_t[Qc][CB:CB + 1, :])  # frees yps
                rinv = work.tile([1, 1024], F32, tag="rinv")
                nc.vector.reciprocal_approx_fast(rinv, ys)
                rb = work.tile([P, 1024], F32, tag="rb")
                nc.gpsimd.partition_broadcast(rb, rinv)
                del yps_t[Qc]
                return (yu, rb)

            def epilogue_b(Qc, yu, rb):
                # OLAG steps later: project + normalize, one 512 half at
                # a time (a matmul's PSUM output must stay in one bank)
                q0 = Qc * 1024
                op = spool.tile([P, 1024], F32, tag="s")
                for v in range(2):
                    vs = slice(v * 512, (v + 1) * 512)
                    ops = op[:, v * 512:(v + 1) * 512]
                    nc.tensor.matmul(ops, wlr, yu[0:CB, vs],
                                     start=True, stop=True)
                    ob = work.tile([P, 512], F32, tag="ob")
                    nc.vector.tensor_mul(ob, ops, rb[:, vs])
                    ob2 = work.tile([P, 512], F32, tag="ob2")
                    nc.gpsimd.tensor_add(
                        ob2, ob, xb[:, q0 + v * 512:q0 + (v + 1) * 512]
                    )
                    nc.sync.dma_start(
                        out=out_d[:, q0 + v * 512:q0 + (v + 1) * 512],
                        in_=ob2,
                    )

            def flush(step):
                while pend_ep and pend_ep[0][0] <= step:
                    _, args = pend_ep.pop(0)
                    epilogue_b(*args)
                if pend_y and step - pend_y[0][0] >= YLAG:
                    _, Qc, i, pbh = pend_y.pop(0)
                    y_mm(Qc, i, pbh)
                    if i == 15:
                        pend_ep.append((step + OLAG, (Qc, *epilogue_a(Qc))))

            for step in range(64):
                Qc, i = divmod(step, 16)
                q0 = Qc * 1024
                if i == 0:
                    yps_t[Qc] = ypool.tile(
                        [CB + 1, 1024], F32, tag="y", name="yps"
                    )
                pbh = []
                pbts = []
                sps = []
                for h in range(2):
                    mi = 2 * i + h
                    if _dve_mtile(mi):
                        pbt = probs.tile([P, 1024], I16, tag="pb", name="pbt")
                        pbh.append(pbt.bitcast(BF16))
                    else:
                        pbt = probs.tile([P, 1024], BF16, tag="pb", name="pbt")
                        pbh.append(pbt)
                    pbts.append(pbt)
                    sps.append(spool.tile([P, 1024], F32, tag="s", name="sp"))
                # interleave h0/h1 so the two streams overlap in disjoint
                # PE row groups; qB reuses qA's stationary weights
                for v in range(2):   # qA / qB 512-halves
                    vs = slice(q0 + v * 512, q0 + (v + 1) * 512)
                    for h in range(2):
                        mi = 2 * i + h
                        half = (slice(0, CB) if h == 0 else slice(CB, P))
                        nc.tensor.matmul(
                            sps[h][:, v * 512:(v + 1) * 512],
                            phi[half, mi * 128:(mi + 1) * 128],
                            theta[half, vs], start=True, stop=True,
                        )
                for h in range(2):
                    mi = 2 * i + h
                    if _dve_mtile(mi):
                        nc.vector.tensor_scalar(
                            pbts[h], sps[h], S16, B16,
                            mybir.AluOpType.mult, mybir.AluOpType.add,
                        )
                    else:
                        nc.scalar.activation(
                            pbts[h], sps[h],
                            mybir.ActivationFunctionType.Exp,
                        )
                pend_y.append((step, Qc, i, pbh))
                flush(step)

            # drain
            step = 64
            while pend_y or pend_ep:
                flush(step)
                step += 1

    nc.finalize()
    return nc


def kernel(x, w_theta, w_phi, w_g, w_last):
    B, C, H, W = x.shape
    N = H * W
    xf = np.ascontiguousarray(x.reshape(B, C, N), dtype=np.float32)
    xh = np.ascontiguousarray(xf, dtype=np.float16)
    wqa = np.ascontiguousarray(
        np.concatenate([w_theta.T, w_theta.T], axis=1), dtype=np.float16
    )
    wqb = np.ascontiguousarray(
        np.concatenate([w_phi.T, w_phi.T], axis=1), dtype=np.float16
    )
    wgT = np.ascontiguousarray(w_g.T, dtype=np.float16)
    wl = np.ascontiguousarray(w_last.T, dtype=np.float32)

    if "nc" not in _NC_CACHE:
        _NC_CACHE["nc"] = _build()
    nc = _NC_CACHE["nc"]

    in_maps = [
        {"xb": xf[b], "xh": xh[b], "wqa": wqa, "wqb": wqb,
         "wgT": wgT, "wl": wl}
        for b in range(B)
    ]
    r = run_bass_kernel_spmd(nc, in_maps, list(range(B)))
    out = np.stack([r.results[b]["out"] for b in range(B)], axis=0)
    return out.reshape(B, C, H, W).astype(np.float32)
